# revision 37
# baseline (speedup 1.0000x reference)
"""Location-sensitive attention Trainium2 kernel (v5.14 — single-copy + hybrid ctx).

Strategy (data-parallel over batch, 8 cores, B=128 -> 16 per core):
  - encoder shipped transposed bf16 [E-part, t] per batch (the projection
    layout).  Only the last 6 of 16 batches ALSO ship the natural layout;
    the other 10 batches' context is computed without it, cutting HBM
    traffic from 16.8 MB (v4 dual-copy) to ~12.3 MB per core.
  - energies matmul doubles as a broadcast: lhsT = W_e replicated across
    all 128 columns -> the energy row lands on PSUM replicated across all
    128 partitions.  exp of that (ACT) is the attention row physically
    present on every partition -- exactly the operand layout the fused
    DVE scalar_tensor_tensor(mult, mult, accum_out) needs to reduce
    ctx[b, e-chunk] = sum_t exp[t] * encT[e, t] per 512-col chunk.
  - ctx engines (load-balanced so PE/DVE/ACT all drain ~equally):
    batches 0-8 DVE STT; batch 9 DVE tensor_tensor premult + ACT
    copy-with-accumulate; batches 10-15 classic PE block-diag matmuls
    against their natural-layout tiles (streamed last, v4-style endgame),
    with each batch's ctx matmuls deferred one batch behind its exp^T
    scatter so the PE never waits on the DVE copy backlog.
  - softmax normalization folded into final copies (scale=1/esum); esum
    from one DVE reduce over the gathered attention rows (the replicated
    exp tiles' partition 0), not from per-exp accumulators.
  - decoder projection decp folded into row 31 (the pad row) of the
    per-batch conv1d stationary: rhs row 31 = ones -> adds decp[a,b].
  - conv1d folded into W_loc on the host (im2col prevrep), b_e dropped
    (softmax shift-invariant).
"""

import sys

for p in ("/opt/trn_rl_repo",):
    if p not in sys.path:
        sys.path.insert(0, p)

import numpy as np
import ml_dtypes

import concourse.bass as bass
import concourse.tile as tile
from concourse import mybir
from concourse import bacc
from concourse import bass_utils
from concourse.masks import make_identity

BF = ml_dtypes.bfloat16

NCORES = 8
B, T, E, D, A, F, KW = 128, 512, 512, 1024, 128, 32, 31
BS = B // NCORES          # 16 batches per core
P = 128

ND = 9                    # batches 0..8: DVE STT ctx
NA = 1                    # batch 9: DVE premult + ACT reduce
NP = 6                    # batches 10..15: PE ctx from natural layout
NDA = ND + NA


def build_device_program(nc):
    dt = mybir.dt
    f32, bf16 = dt.float32, dt.bfloat16
    Act = mybir.ActivationFunctionType
    Alu = mybir.AluOpType

    # [w_encT 4x128 cols | w_e broadcast 128 cols]
    wpack = nc.dram_tensor("wpack", (P, 4 * A + P), bf16, kind="ExternalInput").ap()
    # per-batch conv stationary+moving: [:, b, 0:128] = [Wcomb.T; decp_b],
    # [:, b, 128:640] = [prev windows; ones]
    prevpack = nc.dram_tensor("prevpack", (32, BS, A + T), bf16,
                              kind="ExternalInput").ap()
    encT = nc.dram_tensor("encT", (P, BS, 4, T), bf16, kind="ExternalInput").ap()
    # natural layout, batches 10..15 only: [p, j, q, e] = enc[10+j, q*128+p, e]
    enc_nat = nc.dram_tensor("enc_nat", (P, NP, 4, E), bf16,
                             kind="ExternalInput").ap()
    ctx_out = nc.dram_tensor("context_out", (BS, E), f32, kind="ExternalOutput").ap()
    attn_out = nc.dram_tensor("attn_out", (BS, T), f32, kind="ExternalOutput").ap()

    with tile.TileContext(nc) as tc:
        with (
            tc.tile_pool(name="const", bufs=1) as const,
            tc.tile_pool(name="big", bufs=1) as big,
            tc.tile_pool(name="work", bufs=1) as work,
            tc.tile_pool(name="ps_pe", bufs=2, space="PSUM") as ps_pe,
            tc.tile_pool(name="ps_en", bufs=2, space="PSUM") as ps_en,
            tc.tile_pool(name="ps_at", bufs=1, space="PSUM") as ps_at,
            tc.tile_pool(name="ps_c6", bufs=1, space="PSUM") as ps_c6,
            tc.tile_pool(name="ps_ct", bufs=1, space="PSUM") as ps_ct,
        ):
            wpack_sb = const.tile([P, 4 * A + P], bf16)
            prevpack_sb = const.tile([32, BS, A + T], bf16)
            ident = const.tile([P, P], f32)
            make_identity(nc, ident)
            identb = const.tile([1, 1], bf16)
            nc.vector.memset(identb, 1.0)

            # p-state warmups: the PE clock drops to 1.2 GHz after any idle
            # gap and needs ~3us of continuous work to recover.  These dummy
            # matmuls depend only on the on-chip identity, so they spin the
            # PE at full clock through the DMA ramp until real data lands.
            for _ in range(5):
                wt = ps_en.tile([P, P], f32, tag="energ", bufs=2)
                nc.tensor.matmul(wt, lhsT=ident, rhs=ident, start=True, stop=True)

            # ---- encoder stream: batch 0 split per-chunk for early start ----
            encb0 = [big.tile([P, T], bf16, name=f"encb0c{e}", tag=f"encb0c{e}")
                     for e in range(4)]
            encb = [None] + [big.tile([P, 4, T], bf16, name=f"enc{b}", tag=f"enc{b}")
                             for b in range(1, BS)]
            nat_sb = [big.tile([P, 4, E], bf16, name=f"nat{j}", tag=f"nat{j}")
                      for j in range(NP)]

            # first weight chunk + first encoder chunk go out on the ACT
            # queue, whose preamble finishes before the sync ring's -- the
            # projection's first matmul starts that much earlier
            nc.scalar.dma_start(wpack_sb[:, 0:A], wpack[:, 0:A])
            for q in range(4):
                nc.scalar.dma_start(encb0[0][:, q * P:(q + 1) * P],
                                    encT[:, 0, 0, q * P:(q + 1) * P])
            nc.scalar.dma_start(encb0[1], encT[:, 0, 1])
            nc.sync.dma_start(wpack_sb[:, A:], wpack[:, A:])
            for e in range(2, 4):
                nc.sync.dma_start(encb0[e], encT[:, 0, e])
            # conv pack split: first batches' slice lands before batch 0's
            # conv matmul needs it; the rest follows batch 2's tile
            nc.sync.dma_start(prevpack_sb[:, 0:4, :], prevpack[:, 0:4, :])
            nc.sync.dma_start(encb[1], encT[:, 1])
            nc.sync.dma_start(encb[2], encT[:, 2])
            nc.sync.dma_start(prevpack_sb[:, 4:BS, :], prevpack[:, 4:BS, :])
            for b in range(3, BS):
                nc.sync.dma_start(encb[b], encT[:, b])
            for j in range(NP):
                nc.sync.dma_start(nat_sb[j], enc_nat[:, j])

            def enc_chunk(b, e):
                return encb0[e] if b == 0 else encb[b][:, e, :]

            # ---- persistent result tiles ----
            exp_big = work.tile([P, BS, T], bf16, name="exp_big", tag="exp_big")
            ctxTraw = work.tile([P, 4, NDA], f32, name="ctxTraw", tag="ctxTraw")
            junk_d = work.tile([P, T], bf16, name="junk_d", tag="junk_d")
            junk_a = work.tile([P, T], bf16, name="junk_a", tag="junk_a")
            wenc9 = work.tile([P, 4, T], bf16, name="wenc9", tag="wenc9")
            # block-diag scattered exp^T for the PE-ctx batches
            L6 = work.tile([P, 4 * NP, 8], bf16, name="L6", tag="L6")
            nc.gpsimd.memset(L6, 0.0)

            pe_t = [None] * BS

            def proj4(b):
                pt = ps_pe.tile([A, T], f32, tag="pe", bufs=3)
                pe_t[b] = pt
                if b == 0:
                    # batch 0 accumulates onto a DVE-zeroed bank with
                    # start=False so its first matmul can consume a 32KB
                    # quarter-slice of the still-streaming first chunk
                    nc.vector.memset(pt, 0.0)
                    for q in range(4):
                        nc.tensor.matmul(
                            pt[:, q * P:(q + 1) * P],
                            lhsT=wpack_sb[:, 0:A],
                            rhs=encb0[0][:, q * P:(q + 1) * P],
                            start=False,
                            stop=False,
                            skip_group_check=True,
                        )
                for e in range(0 if b else 1, 4):
                    nc.tensor.matmul(
                        pt,
                        lhsT=wpack_sb[:, e * A:(e + 1) * A],
                        rhs=enc_chunk(b, e),
                        start=(e == 0 and b != 0),
                        stop=False,
                        skip_group_check=(b == 0),
                    )

            def prevconv(b):
                nc.tensor.matmul(
                    pe_t[b],
                    lhsT=prevpack_sb[:, b, 0:A],
                    rhs=prevpack_sb[:, b, A:],
                    start=False,
                    stop=True,
                    skip_group_check=(b == 0),
                )

            def tanh(b):
                th = work.tile([A, T], bf16, name=f"tanh{b}", tag="tanh", bufs=3)
                nc.scalar.activation(th, pe_t[b], Act.Tanh, scale=1.0)
                return th

            tanh_t = [None] * BS
            # bf16 in PSUM must land on 4-byte boundaries: stride the exp^T
            # columns two apart and read back with a strided AP
            psum_at = ps_at.tile([P, 4 * NP, 2], bf16, name="psum_at", tag="attnT")
            psum_c6 = ps_c6.tile([8, E], f32, name="psum_c6", tag="ctx6")

            def energy_exp_ctx(b):
                # energies + broadcast in one matmul: lhsT = we in all cols
                pe_e = ps_en.tile([P, T], f32, tag="energ", bufs=2)
                nc.tensor.matmul(
                    pe_e,
                    lhsT=wpack_sb[:, 4 * A:4 * A + P],
                    rhs=tanh_t[b],
                    start=True,
                    stop=True,
                )
                nc.scalar.activation(exp_big[:, b, :], pe_e, Act.Exp, scale=1.0)
                if b < ND:
                    for e in range(4):
                        nc.vector.scalar_tensor_tensor(
                            out=junk_d,
                            in0=enc_chunk(b, e),
                            scalar=1.0,
                            in1=exp_big[:, b, :],
                            op0=Alu.mult,
                            op1=Alu.mult,
                            accum_out=ctxTraw[:, e, b:b + 1],
                        )
                elif b < NDA:
                    # premult on DVE (2x mode), reduce on ACT
                    for e in range(4):
                        nc.vector.tensor_tensor(
                            out=wenc9[:, e, :],
                            in0=enc_chunk(b, e),
                            in1=exp_big[:, b, :],
                            op=Alu.mult,
                        )
                    for e in range(4):
                        nc.scalar.activation(
                            junk_a, wenc9[:, e, :], Act.Copy,
                            accum_out=ctxTraw[:, e, b:b + 1],
                        )
                else:
                    # PE path: scatter exp^T into L6 via PE transposes.
                    # The ctx matmuls of the PREVIOUS PE batch are issued
                    # here instead, giving the DVE scatter copy a full
                    # batch of slack before the PE consumes L6.
                    j = b - NDA
                    for q in range(4):
                        nc.tensor.transpose(
                            psum_at[:, 4 * j + q, 0:1],
                            exp_big[0:1, b, q * P:(q + 1) * P],
                            identb,
                        )
                    nc.vector.tensor_copy(
                        L6[:, 4 * j:4 * j + 4, j:j + 1],
                        psum_at[:, 4 * j:4 * j + 4, 0:1],
                    )
                    if j >= 1:
                        ctx6_mm(j - 1)
                    if j == NP - 1:
                        ctx6_mm(j)

            def ctx6_mm(j):
                for q in range(4):
                    c = 4 * j + q
                    nc.tensor.matmul(
                        psum_c6,
                        lhsT=L6[:, c, :],
                        rhs=nat_sb[j][:, q, :],
                        start=(c == 0),
                        stop=(c == 4 * NP - 1),
                    )

            def emit_attn_A():
                # group-A attention path: only needs exp rows 0..NDA-1,
                # so it runs in engine-queue slack well before the drain
                nc.sync.dma_start(attn_rawA, exp_big[0:1, 0:NDA, :])
                nc.vector.reduce_sum(esumA, attn_rawA, axis=mybir.AxisListType.X)
                nc.vector.reciprocal(rsA, esumA)
                nc.scalar.activation(attn_nA, attn_rawA, Act.Copy, scale=rsA)
                nc.scalar.dma_start(attn_out[0:NDA], attn_nA)

            # ---- software pipeline.  The conv matmul of batch b is issued
            # after batch b+1's four encoder matmuls: the in-order PE then
            # never stalls on the prevpack transfer (which streams behind
            # batch 0's chunks).  Energies trail by one more batch so they
            # never wait on the ACT tanh backlog. ----
            attn_rawA = work.tile([NDA, T], bf16, name="attn_rawA", tag="attn_rawA")
            esumA = work.tile([NDA, 1], f32, name="esumA", tag="esumA")
            rsA = work.tile([NDA, 1], f32, name="rsA", tag="rsA")
            attn_nA = work.tile([NDA, T], f32, name="attn_nA", tag="attn_nA")
            for b in range(BS):
                proj4(b)
                if b >= 1:
                    prevconv(b - 1)
                    tanh_t[b - 1] = tanh(b - 1)
                if b >= 2:
                    energy_exp_ctx(b - 2)
            prevconv(BS - 1)
            tanh_t[BS - 1] = tanh(BS - 1)
            energy_exp_ctx(BS - 2)
            energy_exp_ctx(BS - 1)
            emit_attn_A()

            ps_ctxT = ps_ct.tile([NDA, 4, P], f32, name="ps_ctxT", tag="ps_ctxT")
            for e in range(4):
                nc.tensor.transpose(ps_ctxT[:, e, :], ctxTraw[:, e, :], ident)
            ctx_a = work.tile([NDA, 4, P], f32, name="ctx_a", tag="ctx_a")
            nc.vector.tensor_scalar_mul(ctx_a, ps_ctxT, rsA)
            nc.sync.dma_start(ctx_out[0:NDA], ctx_a)

            # ---- outputs, group B (PE batches 10..15, the drain tail) ----
            attn_rawB = work.tile([NP, T], bf16, name="attn_rawB", tag="attn_rawB")
            nc.sync.dma_start(attn_rawB, exp_big[0:1, NDA:BS, :])
            esumB = work.tile([NP, 1], f32, name="esumB", tag="esumB")
            nc.vector.reduce_sum(esumB, attn_rawB, axis=mybir.AxisListType.X)
            rsB = work.tile([NP, 1], f32, name="rsB", tag="rsB")
            nc.vector.reciprocal(rsB, esumB)
            attn_nB = work.tile([NP, T], f32, name="attn_nB", tag="attn_nB")
            # attn_nB on DVE (free at drain) and BEFORE ctx_b: it only
            # needs rsB, while ctx_b also waits on the last PE matmul
            nc.vector.tensor_scalar_mul(attn_nB, attn_rawB, rsB)
            # tail outputs ride different queues so the two ~750ns trigger
            # issues overlap instead of serializing on sync
            nc.scalar.dma_start(attn_out[NDA:BS], attn_nB)
            ctx_b = work.tile([NP, E], f32, name="ctx_b", tag="ctx_b")
            nc.vector.tensor_scalar_mul(ctx_b, psum_c6[0:NP, :], rsB)
            nc.sync.dma_start(ctx_out[NDA:BS], ctx_b)

    return nc


def host_prepare(encoder_outputs, decoder_state, prev_attention_weights,
                 W_enc, W_dec, conv_w, W_loc, W_e, b_e):
    """Build per-core input maps (host-side marshaling, all numpy)."""
    f32 = np.float32
    enc = np.asarray(encoder_outputs, dtype=f32)
    dec = np.asarray(decoder_state, dtype=f32)
    prev = np.asarray(prev_attention_weights, dtype=f32)
    W_enc = np.asarray(W_enc, dtype=f32)
    W_dec = np.asarray(W_dec, dtype=f32)
    conv_w = np.asarray(conv_w, dtype=f32)
    W_loc = np.asarray(W_loc, dtype=f32)
    W_e = np.asarray(W_e, dtype=f32)

    # wpack: [p, 4*A] = W_enc.T chunks; [p, 4A:4A+128] = W_e in every column
    wpack = np.zeros((P, 4 * A + P), dtype=BF)
    wpack[:, :4 * A] = (
        W_enc.T.reshape(4, P, A).transpose(1, 0, 2).reshape(P, 4 * A).astype(BF)
    )
    wpack[:, 4 * A:] = W_e[0].astype(BF)[:, None]

    Wcomb = W_loc @ conv_w[:, 0, :]                            # [A, KW]
    pp = np.pad(prev, ((0, 0), (15, 15)))                      # [B, T+30]
    decp_full = (W_dec @ dec.T).astype(f32)                    # [A, B]

    in_maps = []
    for c in range(NCORES):
        sl = slice(c * BS, (c + 1) * BS)
        enc_c = enc[sl].astype(BF)                             # [BS, T, E]
        # encT: [p, b, et, t] = enc[b, t, et*128+p]
        encT = np.ascontiguousarray(
            enc_c.transpose(2, 0, 1)                           # [E, BS, T]
            .reshape(4, P, BS, T)
            .transpose(1, 2, 0, 3)                             # [p, b, et, t]
        )
        # enc_nat: [p, j, q, e] = enc[10+j, q*128+p, e]
        enc_nat = np.ascontiguousarray(
            enc_c[NDA:BS].reshape(NP, 4, P, E).transpose(2, 0, 1, 3)
        )
        prevpack = np.zeros((32, BS, A + T), dtype=BF)
        pc = pp[sl]
        for b in range(BS):
            prevpack[:KW, b, :A] = Wcomb.T.astype(BF)
            prevpack[KW, b, :A] = decp_full[:, c * BS + b].astype(BF)
            for k in range(KW):
                prevpack[k, b, A:] = pc[b, k:k + T].astype(BF)
            prevpack[KW, b, A:] = 1.0
        in_maps.append({
            "encT": encT,
            "enc_nat": enc_nat,
            "wpack": wpack,
            "prevpack": np.ascontiguousarray(prevpack),
        })
    return in_maps


_NC_CACHE = {}


def get_nc():
    if "nc" not in _NC_CACHE:
        nc = bacc.Bacc("TRN2", debug=False, num_devices=NCORES)
        build_device_program(nc)
        nc.finalize()
        _NC_CACHE["nc"] = nc
    return _NC_CACHE["nc"]


def kernel(encoder_outputs, decoder_state, prev_attention_weights,
           W_enc, W_dec, conv_w, W_loc, W_e, b_e, _trace=False, _result_box=None):
    in_maps = host_prepare(
        encoder_outputs, decoder_state, prev_attention_weights,
        W_enc, W_dec, conv_w, W_loc, W_e, b_e,
    )
    nc = get_nc()
    res = bass_utils.run_bass_kernel_spmd(
        nc, in_maps, core_ids=list(range(NCORES)), trace=_trace,
    )
    if _result_box is not None:
        _result_box.append(res)
    ctx = np.concatenate([r["context_out"] for r in res.results], axis=0)
    attn = np.concatenate([r["attn_out"] for r in res.results], axis=0)
    return ctx.astype(np.float32), attn.astype(np.float32)


# revision 39
# speedup vs baseline: 1.0623x; 1.0623x over previous
"""Location-sensitive attention Trainium2 kernel (v5.14 — single-copy + hybrid ctx).

Strategy (data-parallel over batch, 8 cores, B=128 -> 16 per core):
  - encoder shipped transposed bf16 [E-part, t] per batch (the projection
    layout).  Only the last 6 of 16 batches ALSO ship the natural layout;
    the other 10 batches' context is computed without it, cutting HBM
    traffic from 16.8 MB (v4 dual-copy) to ~12.3 MB per core.
  - energies matmul doubles as a broadcast: lhsT = W_e replicated across
    all 128 columns -> the energy row lands on PSUM replicated across all
    128 partitions.  exp of that (ACT) is the attention row physically
    present on every partition -- exactly the operand layout the fused
    DVE scalar_tensor_tensor(mult, mult, accum_out) needs to reduce
    ctx[b, e-chunk] = sum_t exp[t] * encT[e, t] per 512-col chunk.
  - ctx engines (load-balanced so PE/DVE/ACT all drain ~equally):
    batches 0-8 DVE STT; batch 9 DVE tensor_tensor premult + ACT
    copy-with-accumulate; batches 10-15 classic PE block-diag matmuls
    against their natural-layout tiles (streamed last, v4-style endgame),
    with each batch's ctx matmuls deferred one batch behind its exp^T
    scatter so the PE never waits on the DVE copy backlog.
  - softmax normalization folded into final copies (scale=1/esum); esum
    from one DVE reduce over the gathered attention rows (the replicated
    exp tiles' partition 0), not from per-exp accumulators.
  - decoder projection decp folded into row 31 (the pad row) of the
    per-batch conv1d stationary: rhs row 31 = ones -> adds decp[a,b].
  - conv1d folded into W_loc on the host (im2col prevrep), b_e dropped
    (softmax shift-invariant).
"""

import sys

for p in ("/opt/trn_rl_repo",):
    if p not in sys.path:
        sys.path.insert(0, p)

import numpy as np
import ml_dtypes

import concourse.bass as bass
import concourse.tile as tile
from concourse import mybir
from concourse import bacc
from concourse import bass_utils
from concourse.masks import make_identity

BF = ml_dtypes.bfloat16

NCORES = 8
B, T, E, D, A, F, KW = 128, 512, 512, 1024, 128, 32, 31
BS = B // NCORES          # 16 batches per core
P = 128

ND = 9                    # batches 0..8: DVE STT ctx
NA = 1                    # batch 9: DVE premult + ACT reduce
NP = 6                    # batches 10..15: PE ctx from natural layout
NDA = ND + NA


def build_device_program(nc):
    dt = mybir.dt
    f32, bf16 = dt.float32, dt.bfloat16
    Act = mybir.ActivationFunctionType
    Alu = mybir.AluOpType

    # [w_encT 4x128 cols | w_e broadcast 128 cols]
    wpack = nc.dram_tensor("wpack", (P, 4 * A + P), bf16, kind="ExternalInput").ap()
    # per-batch conv stationary+moving: [:, b, 0:128] = [Wcomb.T; decp_b],
    # [:, b, 128:640] = [prev windows; ones]
    prevpack = nc.dram_tensor("prevpack", (32, BS, A + T), bf16,
                              kind="ExternalInput").ap()
    encT = nc.dram_tensor("encT", (P, BS, 4, T), bf16, kind="ExternalInput").ap()
    # natural layout, batches 10..15 only: [p, j, q, e] = enc[10+j, q*128+p, e]
    enc_nat = nc.dram_tensor("enc_nat", (P, NP, 4, E), bf16,
                             kind="ExternalInput").ap()
    ctx_out = nc.dram_tensor("context_out", (BS, E), f32, kind="ExternalOutput").ap()
    attn_out = nc.dram_tensor("attn_out", (BS, T), f32, kind="ExternalOutput").ap()

    with tile.TileContext(nc) as tc:
        with (
            tc.tile_pool(name="const", bufs=1) as const,
            tc.tile_pool(name="big", bufs=1) as big,
            tc.tile_pool(name="work", bufs=1) as work,
            tc.tile_pool(name="ps_pe", bufs=2, space="PSUM") as ps_pe,
            tc.tile_pool(name="ps_en", bufs=2, space="PSUM") as ps_en,
            tc.tile_pool(name="ps_at", bufs=1, space="PSUM") as ps_at,
            tc.tile_pool(name="ps_c6", bufs=1, space="PSUM") as ps_c6,
            tc.tile_pool(name="ps_ct", bufs=1, space="PSUM") as ps_ct,
        ):
            wpack_sb = const.tile([P, 4 * A + P], bf16)
            prevpack_sb = const.tile([32, BS, A + T], bf16)
            ident = const.tile([P, P], f32)
            make_identity(nc, ident)
            identb = const.tile([1, 1], bf16)
            nc.vector.memset(identb, 1.0)

            # p-state warmups: the PE clock drops to 1.2 GHz after any idle
            # gap and needs ~3us of continuous work to recover.  These dummy
            # matmuls depend only on the on-chip identity, so they spin the
            # PE at full clock through the DMA ramp until real data lands.
            for _ in range(14):
                wt = ps_en.tile([P, P], f32, tag="energ", bufs=2)
                nc.tensor.matmul(wt, lhsT=ident, rhs=ident, start=True, stop=True)

            # ---- encoder stream: batch 0 split per-chunk for early start ----
            encb0 = [big.tile([P, T], bf16, name=f"encb0c{e}", tag=f"encb0c{e}")
                     for e in range(4)]
            encb = [None] + [big.tile([P, 4, T], bf16, name=f"enc{b}", tag=f"enc{b}")
                             for b in range(1, BS)]
            nat_sb = [big.tile([P, 4, E], bf16, name=f"nat{j}", tag=f"nat{j}")
                      for j in range(NP)]

            # first weight chunk + first encoder chunk go out on the ACT
            # queue, whose preamble finishes before the sync ring's -- the
            # projection's first matmul starts that much earlier
            nc.scalar.dma_start(wpack_sb[:, 0:A], wpack[:, 0:A])
            nc.scalar.dma_start(encb0[0], encT[:, 0, 0])
            nc.sync.dma_start(wpack_sb[:, A:], wpack[:, A:])
            for e in range(1, 4):
                nc.sync.dma_start(encb0[e], encT[:, 0, e])
            # conv pack split: first batches' slice lands before batch 0's
            # conv matmul needs it; the rest follows batch 2's tile
            nc.sync.dma_start(prevpack_sb[:, 0:4, :], prevpack[:, 0:4, :])
            nc.sync.dma_start(encb[1], encT[:, 1])
            nc.sync.dma_start(encb[2], encT[:, 2])
            nc.sync.dma_start(prevpack_sb[:, 4:BS, :], prevpack[:, 4:BS, :])
            for b in range(3, BS):
                nc.sync.dma_start(encb[b], encT[:, b])
            for j in range(NP):
                nc.sync.dma_start(nat_sb[j], enc_nat[:, j])

            def enc_chunk(b, e):
                return encb0[e] if b == 0 else encb[b][:, e, :]

            # ---- persistent result tiles ----
            exp_big = work.tile([P, BS, T], bf16, name="exp_big", tag="exp_big")
            ctxTraw = work.tile([P, 4, NDA], f32, name="ctxTraw", tag="ctxTraw")
            junk_d = work.tile([P, T], bf16, name="junk_d", tag="junk_d")
            junk_a = work.tile([P, T], bf16, name="junk_a", tag="junk_a")
            wenc9 = work.tile([P, 4, T], bf16, name="wenc9", tag="wenc9")
            # block-diag scattered exp^T for the PE-ctx batches
            L6 = work.tile([P, 4 * NP, 8], bf16, name="L6", tag="L6")
            nc.gpsimd.memset(L6, 0.0)

            pe_t = [None] * BS

            def proj4(b):
                pt = ps_pe.tile([A, T], f32, tag="pe", bufs=3)
                pe_t[b] = pt
                for e in range(4):
                    nc.tensor.matmul(
                        pt,
                        lhsT=wpack_sb[:, e * A:(e + 1) * A],
                        rhs=enc_chunk(b, e),
                        start=(e == 0),
                        stop=False,
                    )

            def prevconv(b):
                nc.tensor.matmul(
                    pe_t[b],
                    lhsT=prevpack_sb[:, b, 0:A],
                    rhs=prevpack_sb[:, b, A:],
                    start=False,
                    stop=True,
                )

            def tanh(b):
                th = work.tile([A, T], bf16, name=f"tanh{b}", tag="tanh", bufs=3)
                nc.scalar.activation(th, pe_t[b], Act.Tanh, scale=1.0)
                return th

            tanh_t = [None] * BS
            # bf16 in PSUM must land on 4-byte boundaries: stride the exp^T
            # columns two apart and read back with a strided AP
            psum_at = ps_at.tile([P, 4 * NP, 2], bf16, name="psum_at", tag="attnT")
            psum_c6 = ps_c6.tile([8, E], f32, name="psum_c6", tag="ctx6")

            def energy_exp_ctx(b):
                # energies + broadcast in one matmul: lhsT = we in all cols
                pe_e = ps_en.tile([P, T], f32, tag="energ", bufs=2)
                nc.tensor.matmul(
                    pe_e,
                    lhsT=wpack_sb[:, 4 * A:4 * A + P],
                    rhs=tanh_t[b],
                    start=True,
                    stop=True,
                )
                nc.scalar.activation(exp_big[:, b, :], pe_e, Act.Exp, scale=1.0)
                if b < ND:
                    for e in range(4):
                        nc.vector.scalar_tensor_tensor(
                            out=junk_d,
                            in0=enc_chunk(b, e),
                            scalar=1.0,
                            in1=exp_big[:, b, :],
                            op0=Alu.mult,
                            op1=Alu.mult,
                            accum_out=ctxTraw[:, e, b:b + 1],
                        )
                elif b < NDA:
                    # premult on DVE (2x mode), reduce on ACT
                    for e in range(4):
                        nc.vector.tensor_tensor(
                            out=wenc9[:, e, :],
                            in0=enc_chunk(b, e),
                            in1=exp_big[:, b, :],
                            op=Alu.mult,
                        )
                    for e in range(4):
                        nc.scalar.activation(
                            junk_a, wenc9[:, e, :], Act.Copy,
                            accum_out=ctxTraw[:, e, b:b + 1],
                        )
                else:
                    # PE path: scatter exp^T into L6 via PE transposes.
                    # The ctx matmuls of the PREVIOUS PE batch are issued
                    # here instead, giving the DVE scatter copy a full
                    # batch of slack before the PE consumes L6.
                    j = b - NDA
                    for q in range(4):
                        nc.tensor.transpose(
                            psum_at[:, 4 * j + q, 0:1],
                            exp_big[0:1, b, q * P:(q + 1) * P],
                            identb,
                        )
                    nc.vector.tensor_copy(
                        L6[:, 4 * j:4 * j + 4, j:j + 1],
                        psum_at[:, 4 * j:4 * j + 4, 0:1],
                    )
                    if j >= 1:
                        ctx6_mm(j - 1)
                    if j == NP - 1:
                        ctx6_mm(j)

            def ctx6_mm(j):
                for q in range(4):
                    c = 4 * j + q
                    nc.tensor.matmul(
                        psum_c6,
                        lhsT=L6[:, c, :],
                        rhs=nat_sb[j][:, q, :],
                        start=(c == 0),
                        stop=(c == 4 * NP - 1),
                    )

            def emit_attn_A():
                # group-A attention path: only needs exp rows 0..NDA-1,
                # so it runs in engine-queue slack well before the drain
                nc.sync.dma_start(attn_rawA, exp_big[0:1, 0:NDA, :])
                nc.vector.reduce_sum(esumA, attn_rawA, axis=mybir.AxisListType.X)
                nc.vector.reciprocal(rsA, esumA)
                nc.scalar.activation(attn_nA, attn_rawA, Act.Copy, scale=rsA)
                nc.scalar.dma_start(attn_out[0:NDA], attn_nA)

            # ---- software pipeline.  The conv matmul of batch b is issued
            # after batch b+1's four encoder matmuls: the in-order PE then
            # never stalls on the prevpack transfer (which streams behind
            # batch 0's chunks).  Energies trail by one more batch so they
            # never wait on the ACT tanh backlog. ----
            attn_rawA = work.tile([NDA, T], bf16, name="attn_rawA", tag="attn_rawA")
            esumA = work.tile([NDA, 1], f32, name="esumA", tag="esumA")
            rsA = work.tile([NDA, 1], f32, name="rsA", tag="rsA")
            attn_nA = work.tile([NDA, T], f32, name="attn_nA", tag="attn_nA")
            for b in range(BS):
                proj4(b)
                if b >= 1:
                    prevconv(b - 1)
                    tanh_t[b - 1] = tanh(b - 1)
                if b >= 2:
                    energy_exp_ctx(b - 2)
            prevconv(BS - 1)
            tanh_t[BS - 1] = tanh(BS - 1)
            energy_exp_ctx(BS - 2)
            energy_exp_ctx(BS - 1)
            emit_attn_A()

            ps_ctxT = ps_ct.tile([NDA, 4, P], f32, name="ps_ctxT", tag="ps_ctxT")
            for e in range(4):
                nc.tensor.transpose(ps_ctxT[:, e, :], ctxTraw[:, e, :], ident)
            ctx_a = work.tile([NDA, 4, P], f32, name="ctx_a", tag="ctx_a")
            nc.vector.tensor_scalar_mul(ctx_a, ps_ctxT, rsA)
            nc.sync.dma_start(ctx_out[0:NDA], ctx_a)

            # ---- outputs, group B (PE batches 10..15, the drain tail) ----
            attn_rawB = work.tile([NP, T], bf16, name="attn_rawB", tag="attn_rawB")
            nc.sync.dma_start(attn_rawB, exp_big[0:1, NDA:BS, :])
            esumB = work.tile([NP, 1], f32, name="esumB", tag="esumB")
            nc.vector.reduce_sum(esumB, attn_rawB, axis=mybir.AxisListType.X)
            rsB = work.tile([NP, 1], f32, name="rsB", tag="rsB")
            nc.vector.reciprocal(rsB, esumB)
            attn_nB = work.tile([NP, T], f32, name="attn_nB", tag="attn_nB")
            # attn_nB on DVE (free at drain) and BEFORE ctx_b: it only
            # needs rsB, while ctx_b also waits on the last PE matmul
            nc.vector.tensor_scalar_mul(attn_nB, attn_rawB, rsB)
            # tail outputs ride different queues so the two ~750ns trigger
            # issues overlap instead of serializing on sync
            nc.scalar.dma_start(attn_out[NDA:BS], attn_nB)
            ctx_b = work.tile([NP, E], f32, name="ctx_b", tag="ctx_b")
            nc.vector.tensor_scalar_mul(ctx_b, psum_c6[0:NP, :], rsB)
            nc.sync.dma_start(ctx_out[NDA:BS], ctx_b)

    return nc


def host_prepare(encoder_outputs, decoder_state, prev_attention_weights,
                 W_enc, W_dec, conv_w, W_loc, W_e, b_e):
    """Build per-core input maps (host-side marshaling, all numpy)."""
    f32 = np.float32
    enc = np.asarray(encoder_outputs, dtype=f32)
    dec = np.asarray(decoder_state, dtype=f32)
    prev = np.asarray(prev_attention_weights, dtype=f32)
    W_enc = np.asarray(W_enc, dtype=f32)
    W_dec = np.asarray(W_dec, dtype=f32)
    conv_w = np.asarray(conv_w, dtype=f32)
    W_loc = np.asarray(W_loc, dtype=f32)
    W_e = np.asarray(W_e, dtype=f32)

    # wpack: [p, 4*A] = W_enc.T chunks; [p, 4A:4A+128] = W_e in every column
    wpack = np.zeros((P, 4 * A + P), dtype=BF)
    wpack[:, :4 * A] = (
        W_enc.T.reshape(4, P, A).transpose(1, 0, 2).reshape(P, 4 * A).astype(BF)
    )
    wpack[:, 4 * A:] = W_e[0].astype(BF)[:, None]

    Wcomb = W_loc @ conv_w[:, 0, :]                            # [A, KW]
    pp = np.pad(prev, ((0, 0), (15, 15)))                      # [B, T+30]
    decp_full = (W_dec @ dec.T).astype(f32)                    # [A, B]

    in_maps = []
    for c in range(NCORES):
        sl = slice(c * BS, (c + 1) * BS)
        enc_c = enc[sl].astype(BF)                             # [BS, T, E]
        # encT: [p, b, et, t] = enc[b, t, et*128+p]
        encT = np.ascontiguousarray(
            enc_c.transpose(2, 0, 1)                           # [E, BS, T]
            .reshape(4, P, BS, T)
            .transpose(1, 2, 0, 3)                             # [p, b, et, t]
        )
        # enc_nat: [p, j, q, e] = enc[10+j, q*128+p, e]
        enc_nat = np.ascontiguousarray(
            enc_c[NDA:BS].reshape(NP, 4, P, E).transpose(2, 0, 1, 3)
        )
        prevpack = np.zeros((32, BS, A + T), dtype=BF)
        pc = pp[sl]
        for b in range(BS):
            prevpack[:KW, b, :A] = Wcomb.T.astype(BF)
            prevpack[KW, b, :A] = decp_full[:, c * BS + b].astype(BF)
            for k in range(KW):
                prevpack[k, b, A:] = pc[b, k:k + T].astype(BF)
            prevpack[KW, b, A:] = 1.0
        in_maps.append({
            "encT": encT,
            "enc_nat": enc_nat,
            "wpack": wpack,
            "prevpack": np.ascontiguousarray(prevpack),
        })
    return in_maps


_NC_CACHE = {}


def get_nc():
    if "nc" not in _NC_CACHE:
        nc = bacc.Bacc("TRN2", debug=False, num_devices=NCORES)
        build_device_program(nc)
        nc.finalize()
        _NC_CACHE["nc"] = nc
    return _NC_CACHE["nc"]


def kernel(encoder_outputs, decoder_state, prev_attention_weights,
           W_enc, W_dec, conv_w, W_loc, W_e, b_e, _trace=False, _result_box=None):
    in_maps = host_prepare(
        encoder_outputs, decoder_state, prev_attention_weights,
        W_enc, W_dec, conv_w, W_loc, W_e, b_e,
    )
    nc = get_nc()
    res = bass_utils.run_bass_kernel_spmd(
        nc, in_maps, core_ids=list(range(NCORES)), trace=_trace,
    )
    if _result_box is not None:
        _result_box.append(res)
    ctx = np.concatenate([r["context_out"] for r in res.results], axis=0)
    attn = np.concatenate([r["attn_out"] for r in res.results], axis=0)
    return ctx.astype(np.float32), attn.astype(np.float32)


# revision 40
# speedup vs baseline: 1.0976x; 1.0333x over previous
"""Location-sensitive attention Trainium2 kernel (v5.14 — single-copy + hybrid ctx).

Strategy (data-parallel over batch, 8 cores, B=128 -> 16 per core):
  - encoder shipped transposed bf16 [E-part, t] per batch (the projection
    layout).  Only the last 6 of 16 batches ALSO ship the natural layout;
    the other 10 batches' context is computed without it, cutting HBM
    traffic from 16.8 MB (v4 dual-copy) to ~12.3 MB per core.
  - energies matmul doubles as a broadcast: lhsT = W_e replicated across
    all 128 columns -> the energy row lands on PSUM replicated across all
    128 partitions.  exp of that (ACT) is the attention row physically
    present on every partition -- exactly the operand layout the fused
    DVE scalar_tensor_tensor(mult, mult, accum_out) needs to reduce
    ctx[b, e-chunk] = sum_t exp[t] * encT[e, t] per 512-col chunk.
  - ctx engines (load-balanced so PE/DVE/ACT all drain ~equally):
    batches 0-8 DVE STT; batch 9 DVE tensor_tensor premult + ACT
    copy-with-accumulate; batches 10-15 classic PE block-diag matmuls
    against their natural-layout tiles (streamed last, v4-style endgame),
    with each batch's ctx matmuls deferred one batch behind its exp^T
    scatter so the PE never waits on the DVE copy backlog.
  - softmax normalization folded into final copies (scale=1/esum); esum
    from one DVE reduce over the gathered attention rows (the replicated
    exp tiles' partition 0), not from per-exp accumulators.
  - decoder projection decp folded into row 31 (the pad row) of the
    per-batch conv1d stationary: rhs row 31 = ones -> adds decp[a,b].
  - conv1d folded into W_loc on the host (im2col prevrep), b_e dropped
    (softmax shift-invariant).
"""

import sys

for p in ("/opt/trn_rl_repo",):
    if p not in sys.path:
        sys.path.insert(0, p)

import numpy as np
import ml_dtypes

import concourse.bass as bass
import concourse.tile as tile
from concourse import mybir
from concourse import bacc
from concourse import bass_utils
from concourse.masks import make_identity

BF = ml_dtypes.bfloat16

NCORES = 8
B, T, E, D, A, F, KW = 128, 512, 512, 1024, 128, 32, 31
BS = B // NCORES          # 16 batches per core
P = 128

ND = 9                    # batches 0..8: DVE STT ctx
NA = 1                    # batch 9: DVE premult + ACT reduce
NP = 6                    # batches 10..15: PE ctx from natural layout
NDA = ND + NA


def build_device_program(nc):
    dt = mybir.dt
    f32, bf16 = dt.float32, dt.bfloat16
    Act = mybir.ActivationFunctionType
    Alu = mybir.AluOpType

    # [w_encT 4x128 cols | w_e broadcast 128 cols]
    wpack = nc.dram_tensor("wpack", (P, 4 * A + P), bf16, kind="ExternalInput").ap()
    # per-batch conv stationary+moving: [:, b, 0:128] = [Wcomb.T; decp_b],
    # [:, b, 128:640] = [prev windows; ones]
    prevpack = nc.dram_tensor("prevpack", (32, BS, A + T), bf16,
                              kind="ExternalInput").ap()
    encT = nc.dram_tensor("encT", (P, BS, 4, T), bf16, kind="ExternalInput").ap()
    # natural layout, batches 10..15 only: [p, j, q, e] = enc[10+j, q*128+p, e]
    enc_nat = nc.dram_tensor("enc_nat", (P, NP, 4, E), bf16,
                             kind="ExternalInput").ap()
    ctx_out = nc.dram_tensor("context_out", (BS, E), f32, kind="ExternalOutput").ap()
    attn_out = nc.dram_tensor("attn_out", (BS, T), f32, kind="ExternalOutput").ap()

    with tile.TileContext(nc) as tc:
        with (
            tc.tile_pool(name="const", bufs=1) as const,
            tc.tile_pool(name="big", bufs=1) as big,
            tc.tile_pool(name="work", bufs=1) as work,
            tc.tile_pool(name="ps_pe", bufs=2, space="PSUM") as ps_pe,
            tc.tile_pool(name="ps_en", bufs=2, space="PSUM") as ps_en,
            tc.tile_pool(name="ps_at", bufs=1, space="PSUM") as ps_at,
            tc.tile_pool(name="ps_c6", bufs=1, space="PSUM") as ps_c6,
            tc.tile_pool(name="ps_ct", bufs=1, space="PSUM") as ps_ct,
        ):
            wpack_sb = const.tile([P, 4 * A + P], bf16)
            prevpack_sb = const.tile([32, BS, A + T], bf16)
            ident = const.tile([P, P], f32)
            make_identity(nc, ident)
            identb = const.tile([1, 1], bf16)
            nc.vector.memset(identb, 1.0)

            # p-state warmups: the PE clock drops to 1.2 GHz after any idle
            # gap and needs ~3us of continuous work to recover.  These dummy
            # matmuls depend only on the on-chip identity, so they spin the
            # PE at full clock through the DMA ramp until real data lands.
            for _ in range(14):
                wt = ps_en.tile([P, P], f32, tag="energ", bufs=2)
                nc.tensor.matmul(wt, lhsT=ident, rhs=ident, start=True, stop=True)

            # ---- encoder stream: batch 0 split per-chunk for early start ----
            encb0 = [big.tile([P, T], bf16, name=f"encb0c{e}", tag=f"encb0c{e}")
                     for e in range(4)]
            encb = [None] + [big.tile([P, 4, T], bf16, name=f"enc{b}", tag=f"enc{b}")
                             for b in range(1, BS)]
            nat_sb = [big.tile([P, 4, E], bf16, name=f"nat{j}", tag=f"nat{j}")
                      for j in range(NP)]

            # first weight chunk + first encoder chunk go out on the ACT
            # queue, whose preamble finishes before the sync ring's -- the
            # projection's first matmul starts that much earlier
            nc.scalar.dma_start(wpack_sb[:, 0:A], wpack[:, 0:A])
            nc.scalar.dma_start(encb0[0], encT[:, 0, 0])
            nc.scalar.dma_start(encb0[1], encT[:, 0, 1])
            nc.sync.dma_start(wpack_sb[:, A:], wpack[:, A:])
            for e in range(2, 4):
                nc.sync.dma_start(encb0[e], encT[:, 0, e])
            # conv pack split: first batches' slice lands before batch 0's
            # conv matmul needs it; the rest follows batch 2's tile
            nc.sync.dma_start(prevpack_sb[:, 0:4, :], prevpack[:, 0:4, :])
            nc.sync.dma_start(encb[1], encT[:, 1])
            nc.sync.dma_start(encb[2], encT[:, 2])
            nc.sync.dma_start(prevpack_sb[:, 4:BS, :], prevpack[:, 4:BS, :])
            for b in range(3, BS):
                nc.sync.dma_start(encb[b], encT[:, b])
            for j in range(NP):
                nc.sync.dma_start(nat_sb[j], enc_nat[:, j])

            def enc_chunk(b, e):
                return encb0[e] if b == 0 else encb[b][:, e, :]

            # ---- persistent result tiles ----
            exp_big = work.tile([P, BS, T], bf16, name="exp_big", tag="exp_big")
            ctxTraw = work.tile([P, 4, NDA], f32, name="ctxTraw", tag="ctxTraw")
            junk_d = work.tile([P, T], bf16, name="junk_d", tag="junk_d")
            junk_a = work.tile([P, T], bf16, name="junk_a", tag="junk_a")
            wenc9 = work.tile([P, 4, T], bf16, name="wenc9", tag="wenc9")
            # block-diag scattered exp^T for the PE-ctx batches
            L6 = work.tile([P, 4 * NP, 8], bf16, name="L6", tag="L6")
            nc.gpsimd.memset(L6, 0.0)

            pe_t = [None] * BS

            def proj4(b):
                pt = ps_pe.tile([A, T], f32, tag="pe", bufs=3)
                pe_t[b] = pt
                for e in range(4):
                    nc.tensor.matmul(
                        pt,
                        lhsT=wpack_sb[:, e * A:(e + 1) * A],
                        rhs=enc_chunk(b, e),
                        start=(e == 0),
                        stop=False,
                    )

            def prevconv(b):
                nc.tensor.matmul(
                    pe_t[b],
                    lhsT=prevpack_sb[:, b, 0:A],
                    rhs=prevpack_sb[:, b, A:],
                    start=False,
                    stop=True,
                )

            def tanh(b):
                th = work.tile([A, T], bf16, name=f"tanh{b}", tag="tanh", bufs=3)
                nc.scalar.activation(th, pe_t[b], Act.Tanh, scale=1.0)
                return th

            tanh_t = [None] * BS
            # bf16 in PSUM must land on 4-byte boundaries: stride the exp^T
            # columns two apart and read back with a strided AP
            psum_at = ps_at.tile([P, 4 * NP, 2], bf16, name="psum_at", tag="attnT")
            psum_c6 = ps_c6.tile([8, E], f32, name="psum_c6", tag="ctx6")

            def energy_exp_ctx(b):
                # energies + broadcast in one matmul: lhsT = we in all cols
                pe_e = ps_en.tile([P, T], f32, tag="energ", bufs=2)
                nc.tensor.matmul(
                    pe_e,
                    lhsT=wpack_sb[:, 4 * A:4 * A + P],
                    rhs=tanh_t[b],
                    start=True,
                    stop=True,
                )
                nc.scalar.activation(exp_big[:, b, :], pe_e, Act.Exp, scale=1.0)
                if b < ND:
                    for e in range(4):
                        nc.vector.scalar_tensor_tensor(
                            out=junk_d,
                            in0=enc_chunk(b, e),
                            scalar=1.0,
                            in1=exp_big[:, b, :],
                            op0=Alu.mult,
                            op1=Alu.mult,
                            accum_out=ctxTraw[:, e, b:b + 1],
                        )
                elif b < NDA:
                    # premult on DVE (2x mode), reduce on ACT
                    for e in range(4):
                        nc.vector.tensor_tensor(
                            out=wenc9[:, e, :],
                            in0=enc_chunk(b, e),
                            in1=exp_big[:, b, :],
                            op=Alu.mult,
                        )
                    for e in range(4):
                        nc.scalar.activation(
                            junk_a, wenc9[:, e, :], Act.Copy,
                            accum_out=ctxTraw[:, e, b:b + 1],
                        )
                else:
                    # PE path: scatter exp^T into L6 via PE transposes.
                    # The ctx matmuls of the PREVIOUS PE batch are issued
                    # here instead, giving the DVE scatter copy a full
                    # batch of slack before the PE consumes L6.
                    j = b - NDA
                    for q in range(4):
                        nc.tensor.transpose(
                            psum_at[:, 4 * j + q, 0:1],
                            exp_big[0:1, b, q * P:(q + 1) * P],
                            identb,
                        )
                    nc.vector.tensor_copy(
                        L6[:, 4 * j:4 * j + 4, j:j + 1],
                        psum_at[:, 4 * j:4 * j + 4, 0:1],
                    )
                    if j >= 1:
                        ctx6_mm(j - 1)
                    if j == NP - 1:
                        ctx6_mm(j)

            def ctx6_mm(j):
                for q in range(4):
                    c = 4 * j + q
                    nc.tensor.matmul(
                        psum_c6,
                        lhsT=L6[:, c, :],
                        rhs=nat_sb[j][:, q, :],
                        start=(c == 0),
                        stop=(c == 4 * NP - 1),
                    )

            def emit_attn_A():
                # group-A attention path: only needs exp rows 0..NDA-1,
                # so it runs in engine-queue slack well before the drain
                nc.sync.dma_start(attn_rawA, exp_big[0:1, 0:NDA, :])
                nc.vector.reduce_sum(esumA, attn_rawA, axis=mybir.AxisListType.X)
                nc.vector.reciprocal(rsA, esumA)
                nc.scalar.activation(attn_nA, attn_rawA, Act.Copy, scale=rsA)
                nc.scalar.dma_start(attn_out[0:NDA], attn_nA)

            # ---- software pipeline.  The conv matmul of batch b is issued
            # after batch b+1's four encoder matmuls: the in-order PE then
            # never stalls on the prevpack transfer (which streams behind
            # batch 0's chunks).  Energies trail by one more batch so they
            # never wait on the ACT tanh backlog. ----
            attn_rawA = work.tile([NDA, T], bf16, name="attn_rawA", tag="attn_rawA")
            esumA = work.tile([NDA, 1], f32, name="esumA", tag="esumA")
            rsA = work.tile([NDA, 1], f32, name="rsA", tag="rsA")
            attn_nA = work.tile([NDA, T], f32, name="attn_nA", tag="attn_nA")
            for b in range(BS):
                proj4(b)
                if b >= 1:
                    prevconv(b - 1)
                    tanh_t[b - 1] = tanh(b - 1)
                if b >= 2:
                    energy_exp_ctx(b - 2)
            prevconv(BS - 1)
            tanh_t[BS - 1] = tanh(BS - 1)
            energy_exp_ctx(BS - 2)
            energy_exp_ctx(BS - 1)
            emit_attn_A()

            ps_ctxT = ps_ct.tile([NDA, 4, P], f32, name="ps_ctxT", tag="ps_ctxT")
            for e in range(4):
                nc.tensor.transpose(ps_ctxT[:, e, :], ctxTraw[:, e, :], ident)
            ctx_a = work.tile([NDA, 4, P], f32, name="ctx_a", tag="ctx_a")
            nc.vector.tensor_scalar_mul(ctx_a, ps_ctxT, rsA)
            nc.sync.dma_start(ctx_out[0:NDA], ctx_a)

            # ---- outputs, group B (PE batches 10..15, the drain tail) ----
            attn_rawB = work.tile([NP, T], bf16, name="attn_rawB", tag="attn_rawB")
            nc.sync.dma_start(attn_rawB, exp_big[0:1, NDA:BS, :])
            esumB = work.tile([NP, 1], f32, name="esumB", tag="esumB")
            nc.vector.reduce_sum(esumB, attn_rawB, axis=mybir.AxisListType.X)
            rsB = work.tile([NP, 1], f32, name="rsB", tag="rsB")
            nc.vector.reciprocal(rsB, esumB)
            attn_nB = work.tile([NP, T], f32, name="attn_nB", tag="attn_nB")
            # attn_nB on DVE (free at drain) and BEFORE ctx_b: it only
            # needs rsB, while ctx_b also waits on the last PE matmul
            nc.vector.tensor_scalar_mul(attn_nB, attn_rawB, rsB)
            # tail outputs ride different queues so the two ~750ns trigger
            # issues overlap instead of serializing on sync
            nc.scalar.dma_start(attn_out[NDA:BS], attn_nB)
            ctx_b = work.tile([NP, E], f32, name="ctx_b", tag="ctx_b")
            nc.vector.tensor_scalar_mul(ctx_b, psum_c6[0:NP, :], rsB)
            nc.sync.dma_start(ctx_out[NDA:BS], ctx_b)

    return nc


def host_prepare(encoder_outputs, decoder_state, prev_attention_weights,
                 W_enc, W_dec, conv_w, W_loc, W_e, b_e):
    """Build per-core input maps (host-side marshaling, all numpy)."""
    f32 = np.float32
    enc = np.asarray(encoder_outputs, dtype=f32)
    dec = np.asarray(decoder_state, dtype=f32)
    prev = np.asarray(prev_attention_weights, dtype=f32)
    W_enc = np.asarray(W_enc, dtype=f32)
    W_dec = np.asarray(W_dec, dtype=f32)
    conv_w = np.asarray(conv_w, dtype=f32)
    W_loc = np.asarray(W_loc, dtype=f32)
    W_e = np.asarray(W_e, dtype=f32)

    # wpack: [p, 4*A] = W_enc.T chunks; [p, 4A:4A+128] = W_e in every column
    wpack = np.zeros((P, 4 * A + P), dtype=BF)
    wpack[:, :4 * A] = (
        W_enc.T.reshape(4, P, A).transpose(1, 0, 2).reshape(P, 4 * A).astype(BF)
    )
    wpack[:, 4 * A:] = W_e[0].astype(BF)[:, None]

    Wcomb = W_loc @ conv_w[:, 0, :]                            # [A, KW]
    pp = np.pad(prev, ((0, 0), (15, 15)))                      # [B, T+30]
    decp_full = (W_dec @ dec.T).astype(f32)                    # [A, B]

    in_maps = []
    for c in range(NCORES):
        sl = slice(c * BS, (c + 1) * BS)
        enc_c = enc[sl].astype(BF)                             # [BS, T, E]
        # encT: [p, b, et, t] = enc[b, t, et*128+p]
        encT = np.ascontiguousarray(
            enc_c.transpose(2, 0, 1)                           # [E, BS, T]
            .reshape(4, P, BS, T)
            .transpose(1, 2, 0, 3)                             # [p, b, et, t]
        )
        # enc_nat: [p, j, q, e] = enc[10+j, q*128+p, e]
        enc_nat = np.ascontiguousarray(
            enc_c[NDA:BS].reshape(NP, 4, P, E).transpose(2, 0, 1, 3)
        )
        prevpack = np.zeros((32, BS, A + T), dtype=BF)
        pc = pp[sl]
        for b in range(BS):
            prevpack[:KW, b, :A] = Wcomb.T.astype(BF)
            prevpack[KW, b, :A] = decp_full[:, c * BS + b].astype(BF)
            for k in range(KW):
                prevpack[k, b, A:] = pc[b, k:k + T].astype(BF)
            prevpack[KW, b, A:] = 1.0
        in_maps.append({
            "encT": encT,
            "enc_nat": enc_nat,
            "wpack": wpack,
            "prevpack": np.ascontiguousarray(prevpack),
        })
    return in_maps


_NC_CACHE = {}


def get_nc():
    if "nc" not in _NC_CACHE:
        nc = bacc.Bacc("TRN2", debug=False, num_devices=NCORES)
        build_device_program(nc)
        nc.finalize()
        _NC_CACHE["nc"] = nc
    return _NC_CACHE["nc"]


def kernel(encoder_outputs, decoder_state, prev_attention_weights,
           W_enc, W_dec, conv_w, W_loc, W_e, b_e, _trace=False, _result_box=None):
    in_maps = host_prepare(
        encoder_outputs, decoder_state, prev_attention_weights,
        W_enc, W_dec, conv_w, W_loc, W_e, b_e,
    )
    nc = get_nc()
    res = bass_utils.run_bass_kernel_spmd(
        nc, in_maps, core_ids=list(range(NCORES)), trace=_trace,
    )
    if _result_box is not None:
        _result_box.append(res)
    ctx = np.concatenate([r["context_out"] for r in res.results], axis=0)
    attn = np.concatenate([r["attn_out"] for r in res.results], axis=0)
    return ctx.astype(np.float32), attn.astype(np.float32)
